# revision 12
# baseline (speedup 1.0000x reference)
import sys

if "/opt/trn_rl_repo" not in sys.path:
    sys.path.insert(0, "/opt/trn_rl_repo")

import numpy as np

N = 3_000_000
NCORES = 8
NPC = N // NCORES          # 375_000 samples per core
PART = 128                 # SBUF partitions
SPP = 2944                 # samples per partition (padded)
NPADPC = PART * SPP        # 376_832
NT = 2                     # tiles per core
K = SPP // NT              # 1472 samples per tile per partition

# All compute in fp16 on DVE (2x-pumped tensor_tensor) + ACT for 1-input
# ops.  GpSimd offload measured as a net loss (TT's second-operand read
# goes through the shared DVE/GpSimd port pair: co-running inflates both
# engines' ops ~+420 ns).
#
# DVE instruction count is cut ~132 -> ~64 per tile by fusing ops across
# component planes with multi-dim access patterns (inner dim stays step-1
# so the fp16 2x mode is kept — verified on HW: [3,K] fused TT = 2450 ns
# = exactly 2x rate).  Only copy-free fusions are used: strided column
# views of F, plane-group sums, stride-0 broadcasts of per-sample scalars.
# ACT-built replication strips were tried and reverted: they moved ~25K
# elems/tile onto ACT and serialized the engines via WAR ping-pong
# (318 us vs 265 us).
#
# Per-partition DRAM layout: [NT][9 planes][K]; F planes row-major
# (plane 3r+c = F_rc), so column views fc[:, c:9:3] are affine.

SQRT02 = 0.4472135954999579  # sqrt(0.2)
SQRT8 = 2.8284271247461903   # sqrt(8)

_cache = {}


def _emit_tile(nc, sp, fc, pc, AL, AF, f16, f32):
    TT = nc.vector.tensor_tensor
    ACT = nc.scalar.activation
    P = PART

    def tile3(name, n, dt=f16):
        return sp.tile([P, n, K], dt, name=name, tag=name, bufs=1)

    sf = tile3("sf", 9)        # F squares; later sqa + S scratch
    pO = tile3("pO", 9)        # product/temp planes
    cAll = tile3("cAll", 6)    # (c00,c11,c22,c01,c02,c12); later That
    aAll = tile3("aAll", 6)    # (a00,a11,a22,a01,a02,a12); later S
    s3 = tile3("s3", 3)
    scal = tile3("scal", 7)    # (t2b, r3, t3, xk, lam, e8a00, e8a01)
    i3f = tile3("i3f", 1, f32)
    t2b, r3, t3, xk, lam, e8a00, e8a01 = (scal[:, i:i + 1, :] for i in range(7))

    def bc(view, n):
        return view.broadcast_to((P, n, K))

    def pl(tile, i, n=1):
        return tile[:, i:i + n, :]

    # ---- C = F^T F --------------------------------------------------------
    for i in range(3):
        ACT(pl(sf, 3 * i, 3), pl(fc, 3 * i, 3), AF.Square)
    # off-diag: per-term product triples via stride-3 column views of F
    TT(pl(pO, 0, 3), fc[:, 0:9:3, :], fc[:, 1:9:3, :], AL.mult)  # c01 terms
    TT(pl(pO, 3, 3), fc[:, 0:9:3, :], fc[:, 2:9:3, :], AL.mult)  # c02 terms
    TT(pl(pO, 6, 3), fc[:, 1:9:3, :], fc[:, 2:9:3, :], AL.mult)  # c12 terms
    TT(s3, pO[:, 0:9:3, :], pO[:, 1:9:3, :], AL.add)
    TT(cAll[:, 3:6, :], s3, pO[:, 2:9:3, :], AL.add)
    # diag: column sums of the squares
    TT(s3, pl(sf, 0, 3), pl(sf, 3, 3), AL.add)
    TT(cAll[:, 0:3, :], s3, pl(sf, 6, 3), AL.add)

    # ---- A = cof(C) -------------------------------------------------------
    ACT(s3, cAll[:, 5:2:-1, :], AF.Square)  # (c12^2, c02^2, c01^2)
    ACT(pl(sf, 6), pl(cAll, 0), AF.Copy, scale=8.0)    # 8 c00 (for t2b)
    TT(pl(pO, 0), pl(cAll, 1), pl(cAll, 2), AL.mult)   # c11 c22
    TT(pl(pO, 1), pl(cAll, 0), pl(cAll, 2), AL.mult)   # c00 c22
    TT(pl(pO, 2), pl(cAll, 0), pl(cAll, 1), AL.mult)   # c00 c11
    TT(aAll[:, 0:3, :], pl(pO, 0, 3), s3, AL.subtract)
    TT(pl(pO, 3), pl(cAll, 4), pl(cAll, 5), AL.mult)   # c02 c12
    TT(pl(pO, 4), pl(cAll, 3), pl(cAll, 5), AL.mult)   # c01 c12
    TT(pl(pO, 5), pl(cAll, 3), pl(cAll, 4), AL.mult)   # c01 c02
    TT(s3, cAll[:, 3:6, :], cAll[:, 2::-1, :], AL.mult)  # (c01c22, c02c11, c12c00)
    TT(aAll[:, 3:6, :], pl(pO, 3, 3), s3, AL.subtract)

    # sqa as soon as A lands: the biggest ACT block, consumed at That diag
    ACT(pl(sf, 0, 6), pl(aAll, 0, 6), AF.Square)       # sqa

    # ---- t2b = 8 c00 + c11 + c22 = 2 I4 (consumed only by gt, late) ------
    TT(pl(pO, 6), pl(sf, 6), pl(cAll, 1), AL.add)
    TT(t2b, pl(pO, 6), pl(cAll, 2), AL.add)

    # ---- I3 = det C (s3 temps so pO stays free for That products) --------
    TT(pl(s3, 0), pl(cAll, 0), pl(aAll, 0), AL.mult)
    TT(pl(s3, 1), pl(cAll, 3), pl(aAll, 3), AL.mult)
    TT(pl(s3, 2), pl(s3, 0), pl(s3, 1), AL.add)
    TT(pl(s3, 0), pl(cAll, 4), pl(aAll, 4), AL.mult)
    # final det add writes the fp32 recip input directly (skips the ACT
    # up-cast hop; mixed-dtype drops this one [1,K] op to 1x, still a win)
    TT(i3f, pl(s3, 2), pl(s3, 0), AL.add)              # i3, fp32

    # That off-diag products that don't need e8a00/r3: they keep DVE busy
    # while ACT runs the e8 scaled copies and the reciprocal resolves.
    TT(pl(pO, 3), pl(aAll, 3), pl(aAll, 1), AL.mult)   # a01 a11
    TT(pl(pO, 4), pl(aAll, 3), pl(aAll, 5), AL.mult)   # a01 a12
    TT(pl(pO, 5), pl(aAll, 1), pl(aAll, 5), AL.mult)   # a11 a12
    TT(pl(pO, 6), pl(aAll, 4), pl(aAll, 5), AL.mult)   # a02 a12
    ACT(e8a00, pl(aAll, 0), AF.Copy, scale=8.0)
    ACT(e8a01, pl(aAll, 3), AF.Copy, scale=8.0)
    nc.vector.reciprocal_approx_fast(i3f, i3f)         # 1/I3, in place
    TT(pl(pO, 7), pl(aAll, 4), pl(aAll, 2), AL.mult)   # a02 a22
    TT(pl(pO, 8), pl(aAll, 5), pl(aAll, 2), AL.mult)   # a12 a22
    TT(pl(pO, 0), e8a00, pl(aAll, 3), AL.mult)         # 8 a00 a01
    TT(pl(pO, 1), e8a00, pl(aAll, 4), AL.mult)         # 8 a00 a02
    TT(pl(pO, 2), e8a01, pl(aAll, 4), AL.mult)         # 8 a01 a02

    # ---- t3 = 2 I5, kappa (xk), lambda (lam) -----------------------------
    # scalar_tensor_tensor (1x-only, fine at [1,K]) fuses the -56 bias and
    # the -0.2 scale and reads the fp32 reciprocal directly, cutting three
    # serial ACT hops out of the critical path.
    STT = nc.vector.scalar_tensor_tensor
    TT(pl(s3, 0), e8a00, pl(aAll, 1), AL.add)
    TT(t3, pl(s3, 0), pl(aAll, 2), AL.add)
    ACT(pl(s3, 0), t3, AF.Square, scale=SQRT02)        # 0.2 t3^2
    STT(pl(sf, 6), pl(s3, 0), -56.0, i3f, AL.add, AL.mult)   # (0.2t3^2-56) r3
    ACT(xk, pl(sf, 6), AF.Copy, bias=20.0)
    STT(lam, t3, -0.2, i3f, AL.mult, AL.mult)          # -0.2 t3 r3

    # diag 8 A_i0^2 prefetch, then the bulk sqa
    ACT(pl(s3, 0), pl(aAll, 0), AF.Square, scale=SQRT8)   # 8 a00^2
    ACT(pl(s3, 1), pl(aAll, 3), AF.Square, scale=SQRT8)   # 8 a01^2
    ACT(pl(s3, 2), pl(aAll, 4), AF.Square, scale=SQRT8)   # 8 a02^2
    ACT(pl(sf, 0, 6), pl(aAll, 0, 6), AF.Square)       # sqa

    # ---- That accumulation (into cAll slots; C is dead; sf[6:9] is the
    # temp since s3 now carries the th8sq prefetch) -------------------------
    TT(sf[:, 6:9, :], pl(pO, 0, 3), pl(pO, 3, 3), AL.add)
    TT(cAll[:, 3:6, :], sf[:, 6:9, :], pl(pO, 6, 3), AL.add)  # (th01,th02,th12)
    # diag: 8 A_i0^2 + A_i1^2 + A_i2^2 from sqa + scaled squares
    for (i, q1, q2) in ((0, 3, 4), (1, 1, 5), (2, 5, 2)):
        TT(pl(pO, 0), pl(s3, i), pl(sf, q1), AL.add)
        TT(pl(cAll, i), pl(pO, 0), pl(sf, q2), AL.add)

    # ---- S = xk A + lam That + diag(g) -----------------------------------
    TT(pl(sf, 0, 6), pl(aAll, 0, 6), bc(xk, 6), AL.mult)    # k1 (sqa dead)
    TT(pl(pO, 0, 6), pl(cAll, 0, 6), bc(lam, 6), AL.mult)   # k2
    TT(pl(aAll, 0, 6), pl(sf, 0, 6), pl(pO, 0, 6), AL.add)  # S -> aAll
    ACT(pl(s3, 0), t2b, AF.Copy, scale=1.6, bias=16.0)      # g0
    ACT(pl(s3, 1, 2), bc(t2b, 2), AF.Copy, scale=0.2, bias=16.0)  # g12
    TT(pl(sf, 0, 3), pl(aAll, 0, 3), s3, AL.add)            # S diag + g

    # ---- P = F S  (S symmetric; diag in sf[0:3], off-diag in aAll[3:6]) --
    Sv = [[pl(sf, 0), pl(aAll, 3), pl(aAll, 4)],
          [pl(aAll, 3), pl(sf, 1), pl(aAll, 5)],
          [pl(aAll, 4), pl(aAll, 5), pl(sf, 2)]]
    for j in range(3):
        TT(pl(pO, 0, 3), fc[:, 0:9:3, :], bc(Sv[0][j], 3), AL.mult)
        TT(pl(pO, 3, 3), fc[:, 1:9:3, :], bc(Sv[1][j], 3), AL.mult)
        TT(pl(pO, 6, 3), pl(pO, 0, 3), pl(pO, 3, 3), AL.add)
        TT(pl(pO, 0, 3), fc[:, 2:9:3, :], bc(Sv[2][j], 3), AL.mult)
        TT(pc[:, j:9:3, :], pl(pO, 6, 3), pl(pO, 0, 3), AL.add)


def _build():
    import concourse.bass as bass
    import concourse.tile as tile
    from concourse import bacc, mybir
    from contextlib import ExitStack

    f16 = mybir.dt.float16
    f32 = mybir.dt.float32
    AL = mybir.AluOpType
    AF = mybir.ActivationFunctionType

    nc = bacc.Bacc("TRN2", target_bir_lowering=False, debug=False)
    fin_d = nc.dram_tensor("fin", [PART, NT, 9, K], f16, kind="ExternalInput").ap()
    pout_d = nc.dram_tensor("pout", [PART, NT, 9, K], f16, kind="ExternalOutput").ap()

    with tile.TileContext(nc) as tc:
        with ExitStack() as ctx:
            io = ctx.enter_context(tc.tile_pool(name="io", bufs=2))
            sp = ctx.enter_context(tc.tile_pool(name="sp", bufs=1))

            # Issue all input DMAs up front: the tile-t+1 load must not queue
            # behind the tile-t store's semaphore wait on the SP sequencer.
            # Column-group granularity so the first C product (cols 0,1)
            # starts before the full tile has landed.
            fcs = []
            for t in range(NT):
                ft = io.tile([PART, 9, K], f16, name="fin", tag="fin", bufs=2)
                for c in range(3):
                    nc.sync.dma_start(ft[:, c:9:3, :], fin_d[:, t, c:9:3, :])
                fcs.append(ft)

            for t in range(NT):
                pc = io.tile([PART, 9, K], f16, name="pout", tag="pout",
                             bufs=1)
                _emit_tile(nc, sp, fcs[t], pc, AL, AF, f16, f32)
                # per-column stores: P column j is complete as soon as its
                # FS pass finishes, so the tail is one column, not the tile
                for j in range(3):
                    nc.sync.dma_start(pout_d[:, t, j:9:3, :], pc[:, j:9:3, :])

    nc.compile()
    return nc


def _get_nc():
    if "nc" not in _cache:
        _cache["nc"] = _build()
    return _cache["nc"]


def _make_in_maps(F):
    x = F.reshape(N, 9).astype(np.float16)
    eye9 = np.array([1, 0, 0, 0, 1, 0, 0, 0, 1], dtype=np.float16)
    pad = np.tile(eye9, (NPADPC - NPC, 1))
    in_maps = []
    for cidx in range(NCORES):
        xc = x[cidx * NPC:(cidx + 1) * NPC]
        xcp = (np.concatenate([xc, pad], axis=0)
               .reshape(PART, NT, K, 9).transpose(0, 1, 3, 2))
        in_maps.append({"fin": np.ascontiguousarray(xcp)})
    return in_maps


def kernel(**inputs):
    from concourse.bass_utils import run_bass_kernel_spmd

    F = np.asarray(inputs["F"], dtype=np.float32)
    nc = _get_nc()
    in_maps = _make_in_maps(F)

    res = run_bass_kernel_spmd(nc, in_maps, list(range(NCORES)))

    out = np.empty((N, 9), dtype=np.float32)
    for cidx in range(NCORES):
        oc = (np.asarray(res.results[cidx]["pout"]).astype(np.float32)
              .reshape(PART, NT, 9, K).transpose(0, 1, 3, 2)
              .reshape(NPADPC, 9))
        out[cidx * NPC:(cidx + 1) * NPC] = oc[:NPC]
    return out.reshape(N, 3, 3)


# revision 13
# speedup vs baseline: 1.0258x; 1.0258x over previous
import sys

if "/opt/trn_rl_repo" not in sys.path:
    sys.path.insert(0, "/opt/trn_rl_repo")

import numpy as np

N = 3_000_000
NCORES = 8
NPC = N // NCORES          # 375_000 samples per core
PART = 128                 # SBUF partitions
SPP = 2944                 # samples per partition (padded)
NPADPC = PART * SPP        # 376_832
NT = 2                     # tiles per core
K = SPP // NT              # 1472 samples per tile per partition

# All compute in fp16 on DVE (2x-pumped tensor_tensor) + ACT for 1-input
# ops.  GpSimd offload measured as a net loss (TT's second-operand read
# goes through the shared DVE/GpSimd port pair: co-running inflates both
# engines' ops ~+420 ns).
#
# DVE instruction count is cut ~132 -> ~64 per tile by fusing ops across
# component planes with multi-dim access patterns (inner dim stays step-1
# so the fp16 2x mode is kept — verified on HW: [3,K] fused TT = 2450 ns
# = exactly 2x rate).  Only copy-free fusions are used: strided column
# views of F, plane-group sums, stride-0 broadcasts of per-sample scalars.
# ACT-built replication strips were tried and reverted: they moved ~25K
# elems/tile onto ACT and serialized the engines via WAR ping-pong
# (318 us vs 265 us).
#
# Per-partition DRAM layout: [NT][9 planes][K]; F planes row-major
# (plane 3r+c = F_rc), so column views fc[:, c:9:3] are affine.

SQRT02 = 0.4472135954999579  # sqrt(0.2)
SQRT8 = 2.8284271247461903   # sqrt(8)

_cache = {}


def _emit_tile(nc, sp, fc, pc, AL, AF, f16, f32):
    TT = nc.vector.tensor_tensor
    ACT = nc.scalar.activation
    P = PART

    def tile3(name, n, dt=f16):
        return sp.tile([P, n, K], dt, name=name, tag=name, bufs=1)

    sf = tile3("sf", 9)        # F squares; later sqa + S scratch
    pO = tile3("pO", 9)        # product/temp planes
    cAll = tile3("cAll", 6)    # (c00,c11,c22,c01,c02,c12); later That
    aAll = tile3("aAll", 6)    # (a00,a11,a22,a01,a02,a12); later S
    s3 = tile3("s3", 3)
    scal = tile3("scal", 7)    # (t2b, r3, t3, xk, lam, e8a00, e8a01)
    i3f = tile3("i3f", 1, f32)
    t2b, r3, t3, xk, lam, e8a00, e8a01 = (scal[:, i:i + 1, :] for i in range(7))

    def bc(view, n):
        return view.broadcast_to((P, n, K))

    def pl(tile, i, n=1):
        return tile[:, i:i + n, :]

    # ---- C = F^T F --------------------------------------------------------
    for i in range(3):
        ACT(pl(sf, 3 * i, 3), pl(fc, 3 * i, 3), AF.Square)
    # off-diag: per-term product triples via stride-3 column views of F
    TT(pl(pO, 0, 3), fc[:, 0:9:3, :], fc[:, 1:9:3, :], AL.mult)  # c01 terms
    TT(pl(pO, 3, 3), fc[:, 0:9:3, :], fc[:, 2:9:3, :], AL.mult)  # c02 terms
    TT(pl(pO, 6, 3), fc[:, 1:9:3, :], fc[:, 2:9:3, :], AL.mult)  # c12 terms
    TT(s3, pO[:, 0:9:3, :], pO[:, 1:9:3, :], AL.add)
    TT(cAll[:, 3:6, :], s3, pO[:, 2:9:3, :], AL.add)
    # diag: column sums of the squares
    TT(s3, pl(sf, 0, 3), pl(sf, 3, 3), AL.add)
    TT(cAll[:, 0:3, :], s3, pl(sf, 6, 3), AL.add)

    # ---- t2b = 8 c00 + c11 + c22 = 2 I4 ----------------------------------
    ACT(pl(s3, 0), pl(cAll, 0), AF.Copy, scale=8.0)
    TT(pl(pO, 0), pl(s3, 0), pl(cAll, 1), AL.add)
    TT(t2b, pl(pO, 0), pl(cAll, 2), AL.add)

    # ---- A = cof(C) -------------------------------------------------------
    ACT(s3, cAll[:, 5:2:-1, :], AF.Square)  # (c12^2, c02^2, c01^2)
    TT(pl(pO, 0), pl(cAll, 1), pl(cAll, 2), AL.mult)   # c11 c22
    TT(pl(pO, 1), pl(cAll, 0), pl(cAll, 2), AL.mult)   # c00 c22
    TT(pl(pO, 2), pl(cAll, 0), pl(cAll, 1), AL.mult)   # c00 c11
    TT(aAll[:, 0:3, :], pl(pO, 0, 3), s3, AL.subtract)
    TT(pl(pO, 3), pl(cAll, 4), pl(cAll, 5), AL.mult)   # c02 c12
    TT(pl(pO, 4), pl(cAll, 3), pl(cAll, 5), AL.mult)   # c01 c12
    TT(pl(pO, 5), pl(cAll, 3), pl(cAll, 4), AL.mult)   # c01 c02
    TT(s3, cAll[:, 3:6, :], cAll[:, 2::-1, :], AL.mult)  # (c01c22, c02c11, c12c00)
    TT(aAll[:, 3:6, :], pl(pO, 3, 3), s3, AL.subtract)

    # ---- I3 = det C (s3 temps so pO stays free for That products) --------
    TT(pl(s3, 0), pl(cAll, 0), pl(aAll, 0), AL.mult)
    TT(pl(s3, 1), pl(cAll, 3), pl(aAll, 3), AL.mult)
    TT(pl(s3, 2), pl(s3, 0), pl(s3, 1), AL.add)
    TT(pl(s3, 0), pl(cAll, 4), pl(aAll, 4), AL.mult)
    # final det add writes the fp32 recip input directly (skips the ACT
    # up-cast hop; mixed-dtype drops this one [1,K] op to 1x, still a win)
    TT(i3f, pl(s3, 2), pl(s3, 0), AL.add)              # i3, fp32

    # That off-diag products that don't need e8a00/r3: they keep DVE busy
    # while ACT runs the e8 scaled copies and the reciprocal resolves.
    TT(pl(pO, 3), pl(aAll, 3), pl(aAll, 1), AL.mult)   # a01 a11
    TT(pl(pO, 4), pl(aAll, 3), pl(aAll, 5), AL.mult)   # a01 a12
    TT(pl(pO, 5), pl(aAll, 1), pl(aAll, 5), AL.mult)   # a11 a12
    TT(pl(pO, 6), pl(aAll, 4), pl(aAll, 5), AL.mult)   # a02 a12
    ACT(e8a00, pl(aAll, 0), AF.Copy, scale=8.0)
    ACT(e8a01, pl(aAll, 3), AF.Copy, scale=8.0)
    nc.vector.reciprocal_approx_fast(i3f, i3f)         # 1/I3, in place
    TT(pl(pO, 7), pl(aAll, 4), pl(aAll, 2), AL.mult)   # a02 a22
    TT(pl(pO, 8), pl(aAll, 5), pl(aAll, 2), AL.mult)   # a12 a22
    TT(pl(pO, 0), e8a00, pl(aAll, 3), AL.mult)         # 8 a00 a01
    TT(pl(pO, 1), e8a00, pl(aAll, 4), AL.mult)         # 8 a00 a02
    TT(pl(pO, 2), e8a01, pl(aAll, 4), AL.mult)         # 8 a01 a02

    # ---- t3 = 2 I5, kappa (xk), lambda (lam) -----------------------------
    # scalar_tensor_tensor (1x-only, fine at [1,K]) fuses the -56 bias and
    # the -0.2 scale and reads the fp32 reciprocal directly, cutting three
    # serial ACT hops out of the critical path.
    STT = nc.vector.scalar_tensor_tensor
    TT(pl(s3, 0), e8a00, pl(aAll, 1), AL.add)
    TT(t3, pl(s3, 0), pl(aAll, 2), AL.add)
    ACT(pl(s3, 0), t3, AF.Square, scale=SQRT02)        # 0.2 t3^2
    STT(pl(sf, 6), pl(s3, 0), -56.0, i3f, AL.add, AL.mult)   # (0.2t3^2-56) r3
    ACT(xk, pl(sf, 6), AF.Copy, bias=20.0)
    STT(lam, t3, -0.2, i3f, AL.mult, AL.mult)          # -0.2 t3 r3

    # diag 8 A_i0^2 prefetch, then the bulk sqa
    ACT(pl(s3, 0), pl(aAll, 0), AF.Square, scale=SQRT8)   # 8 a00^2
    ACT(pl(s3, 1), pl(aAll, 3), AF.Square, scale=SQRT8)   # 8 a01^2
    ACT(pl(s3, 2), pl(aAll, 4), AF.Square, scale=SQRT8)   # 8 a02^2
    ACT(pl(sf, 0, 6), pl(aAll, 0, 6), AF.Square)       # sqa

    # ---- That accumulation (into cAll slots; C is dead; sf[6:9] is the
    # temp since s3 now carries the th8sq prefetch) -------------------------
    TT(sf[:, 6:9, :], pl(pO, 0, 3), pl(pO, 3, 3), AL.add)
    TT(cAll[:, 3:6, :], sf[:, 6:9, :], pl(pO, 6, 3), AL.add)  # (th01,th02,th12)
    # diag: 8 A_i0^2 + A_i1^2 + A_i2^2 from sqa + scaled squares
    for (i, q1, q2) in ((0, 3, 4), (1, 1, 5), (2, 5, 2)):
        TT(pl(pO, 0), pl(s3, i), pl(sf, q1), AL.add)
        TT(pl(cAll, i), pl(pO, 0), pl(sf, q2), AL.add)

    # ---- S = xk A + lam That + diag(g) -----------------------------------
    TT(pl(sf, 0, 6), pl(aAll, 0, 6), bc(xk, 6), AL.mult)    # k1 (sqa dead)
    TT(pl(pO, 0, 6), pl(cAll, 0, 6), bc(lam, 6), AL.mult)   # k2
    TT(pl(aAll, 0, 6), pl(sf, 0, 6), pl(pO, 0, 6), AL.add)  # S -> aAll
    ACT(pl(s3, 0), t2b, AF.Copy, scale=1.6, bias=16.0)      # g0
    ACT(pl(s3, 1, 2), bc(t2b, 2), AF.Copy, scale=0.2, bias=16.0)  # g12
    TT(pl(sf, 0, 3), pl(aAll, 0, 3), s3, AL.add)            # S diag + g

    # ---- P = F S  (S symmetric; diag in sf[0:3], off-diag in aAll[3:6]) --
    Sv = [[pl(sf, 0), pl(aAll, 3), pl(aAll, 4)],
          [pl(aAll, 3), pl(sf, 1), pl(aAll, 5)],
          [pl(aAll, 4), pl(aAll, 5), pl(sf, 2)]]
    for j in range(3):
        TT(pl(pO, 0, 3), fc[:, 0:9:3, :], bc(Sv[0][j], 3), AL.mult)
        TT(pl(pO, 3, 3), fc[:, 1:9:3, :], bc(Sv[1][j], 3), AL.mult)
        TT(pl(pO, 6, 3), pl(pO, 0, 3), pl(pO, 3, 3), AL.add)
        TT(pl(pO, 0, 3), fc[:, 2:9:3, :], bc(Sv[2][j], 3), AL.mult)
        TT(pc[:, j:9:3, :], pl(pO, 6, 3), pl(pO, 0, 3), AL.add)


def _build():
    import concourse.bass as bass
    import concourse.tile as tile
    from concourse import bacc, mybir
    from contextlib import ExitStack

    f16 = mybir.dt.float16
    f32 = mybir.dt.float32
    AL = mybir.AluOpType
    AF = mybir.ActivationFunctionType

    nc = bacc.Bacc("TRN2", target_bir_lowering=False, debug=False)
    fin_d = nc.dram_tensor("fin", [PART, NT, 9, K], f16, kind="ExternalInput").ap()
    pout_d = nc.dram_tensor("pout", [PART, NT, 9, K], f16, kind="ExternalOutput").ap()

    with tile.TileContext(nc) as tc:
        with ExitStack() as ctx:
            io = ctx.enter_context(tc.tile_pool(name="io", bufs=2))
            sp = ctx.enter_context(tc.tile_pool(name="sp", bufs=1))

            # Issue all input DMAs up front: the tile-t+1 load must not queue
            # behind the tile-t store's semaphore wait on the SP sequencer.
            # Column-group granularity so the first C product (cols 0,1)
            # starts before the full tile has landed.
            fcs = []
            for t in range(NT):
                ft = io.tile([PART, 9, K], f16, name="fin", tag="fin", bufs=2)
                for c in range(3):
                    nc.sync.dma_start(ft[:, c:9:3, :], fin_d[:, t, c:9:3, :])
                fcs.append(ft)

            for t in range(NT):
                pc = io.tile([PART, 9, K], f16, name="pout", tag="pout",
                             bufs=1)
                _emit_tile(nc, sp, fcs[t], pc, AL, AF, f16, f32)
                # per-column stores: P column j is complete as soon as its
                # FS pass finishes, so the tail is one column, not the tile
                for j in range(3):
                    nc.sync.dma_start(pout_d[:, t, j:9:3, :], pc[:, j:9:3, :])

    nc.compile()
    return nc


def _get_nc():
    if "nc" not in _cache:
        _cache["nc"] = _build()
    return _cache["nc"]


def _make_in_maps(F):
    x = F.reshape(N, 9).astype(np.float16)
    eye9 = np.array([1, 0, 0, 0, 1, 0, 0, 0, 1], dtype=np.float16)
    pad = np.tile(eye9, (NPADPC - NPC, 1))
    in_maps = []
    for cidx in range(NCORES):
        xc = x[cidx * NPC:(cidx + 1) * NPC]
        xcp = (np.concatenate([xc, pad], axis=0)
               .reshape(PART, NT, K, 9).transpose(0, 1, 3, 2))
        in_maps.append({"fin": np.ascontiguousarray(xcp)})
    return in_maps


def kernel(**inputs):
    from concourse.bass_utils import run_bass_kernel_spmd

    F = np.asarray(inputs["F"], dtype=np.float32)
    nc = _get_nc()
    in_maps = _make_in_maps(F)

    res = run_bass_kernel_spmd(nc, in_maps, list(range(NCORES)))

    out = np.empty((N, 9), dtype=np.float32)
    for cidx in range(NCORES):
        oc = (np.asarray(res.results[cidx]["pout"]).astype(np.float32)
              .reshape(PART, NT, 9, K).transpose(0, 1, 3, 2)
              .reshape(NPADPC, 9))
        out[cidx * NPC:(cidx + 1) * NPC] = oc[:NPC]
    return out.reshape(N, 3, 3)


# revision 14
# speedup vs baseline: 1.0291x; 1.0032x over previous
import sys

if "/opt/trn_rl_repo" not in sys.path:
    sys.path.insert(0, "/opt/trn_rl_repo")

import numpy as np

N = 3_000_000
NCORES = 8
NPC = N // NCORES          # 375_000 samples per core
PART = 128                 # SBUF partitions
SPP = 2944                 # samples per partition (padded)
NPADPC = PART * SPP        # 376_832
NT = 2                     # tiles per core
K = SPP // NT              # 1472 samples per tile per partition

# All compute in fp16 on DVE (2x-pumped tensor_tensor) + ACT for 1-input
# ops.  GpSimd offload measured as a net loss (TT's second-operand read
# goes through the shared DVE/GpSimd port pair: co-running inflates both
# engines' ops ~+420 ns).
#
# DVE instruction count is cut ~132 -> ~64 per tile by fusing ops across
# component planes with multi-dim access patterns (inner dim stays step-1
# so the fp16 2x mode is kept — verified on HW: [3,K] fused TT = 2450 ns
# = exactly 2x rate).  Only copy-free fusions are used: strided column
# views of F, plane-group sums, stride-0 broadcasts of per-sample scalars.
# ACT-built replication strips were tried and reverted: they moved ~25K
# elems/tile onto ACT and serialized the engines via WAR ping-pong
# (318 us vs 265 us).
#
# Per-partition DRAM layout: [NT][9 planes][K]; F planes row-major
# (plane 3r+c = F_rc), so column views fc[:, c:9:3] are affine.

SQRT02 = 0.4472135954999579  # sqrt(0.2)
SQRT8 = 2.8284271247461903   # sqrt(8)

_cache = {}


def _emit_tile(nc, sp, fc, pc, AL, AF, f16, f32):
    TT = nc.vector.tensor_tensor
    ACT = nc.scalar.activation
    P = PART

    def tile3(name, n, dt=f16):
        return sp.tile([P, n, K], dt, name=name, tag=name, bufs=1)

    sf = tile3("sf", 9)        # F squares; later sqa + S scratch
    pO = tile3("pO", 9)        # product/temp planes
    cAll = tile3("cAll", 6)    # (c00,c11,c22,c01,c02,c12); later That
    aAll = tile3("aAll", 6)    # (a00,a11,a22,a01,a02,a12); later S
    s3 = tile3("s3", 3)
    scal = tile3("scal", 7)    # (t2b, r3, t3, xk, lam, e8a00, e8a01)
    i3f = tile3("i3f", 1, f32)
    t2b, r3, t3, xk, lam, e8a00, e8a01 = (scal[:, i:i + 1, :] for i in range(7))

    def bc(view, n):
        return view.broadcast_to((P, n, K))

    def pl(tile, i, n=1):
        return tile[:, i:i + n, :]

    # ---- C = F^T F --------------------------------------------------------
    # squares per COLUMN group: each ACT op only gates on its own column
    # DMA, so the ACT pipeline starts ~4 us earlier than row-group order
    for c in range(3):
        ACT(sf[:, c:9:3, :], fc[:, c:9:3, :], AF.Square)
    # off-diag: per-term product triples via stride-3 column views of F
    TT(pl(pO, 0, 3), fc[:, 0:9:3, :], fc[:, 1:9:3, :], AL.mult)  # c01 terms
    TT(pl(pO, 3, 3), fc[:, 0:9:3, :], fc[:, 2:9:3, :], AL.mult)  # c02 terms
    TT(pl(pO, 6, 3), fc[:, 1:9:3, :], fc[:, 2:9:3, :], AL.mult)  # c12 terms
    TT(s3, pO[:, 0:9:3, :], pO[:, 1:9:3, :], AL.add)
    TT(cAll[:, 3:6, :], s3, pO[:, 2:9:3, :], AL.add)
    # diag: column sums of the squares
    TT(s3, pl(sf, 0, 3), pl(sf, 3, 3), AL.add)
    TT(cAll[:, 0:3, :], s3, pl(sf, 6, 3), AL.add)

    # ---- t2b = 8 c00 + c11 + c22 = 2 I4 ----------------------------------
    ACT(pl(s3, 0), pl(cAll, 0), AF.Copy, scale=8.0)
    TT(pl(pO, 0), pl(s3, 0), pl(cAll, 1), AL.add)
    TT(t2b, pl(pO, 0), pl(cAll, 2), AL.add)

    # ---- A = cof(C) -------------------------------------------------------
    ACT(s3, cAll[:, 5:2:-1, :], AF.Square)  # (c12^2, c02^2, c01^2)
    TT(pl(pO, 0), pl(cAll, 1), pl(cAll, 2), AL.mult)   # c11 c22
    TT(pl(pO, 1), pl(cAll, 0), pl(cAll, 2), AL.mult)   # c00 c22
    TT(pl(pO, 2), pl(cAll, 0), pl(cAll, 1), AL.mult)   # c00 c11
    TT(aAll[:, 0:3, :], pl(pO, 0, 3), s3, AL.subtract)
    TT(pl(pO, 3), pl(cAll, 4), pl(cAll, 5), AL.mult)   # c02 c12
    TT(pl(pO, 4), pl(cAll, 3), pl(cAll, 5), AL.mult)   # c01 c12
    TT(pl(pO, 5), pl(cAll, 3), pl(cAll, 4), AL.mult)   # c01 c02
    TT(s3, cAll[:, 3:6, :], cAll[:, 2::-1, :], AL.mult)  # (c01c22, c02c11, c12c00)
    TT(aAll[:, 3:6, :], pl(pO, 3, 3), s3, AL.subtract)

    # ---- I3 = det C (s3 temps so pO stays free for That products) --------
    TT(pl(s3, 0), pl(cAll, 0), pl(aAll, 0), AL.mult)
    TT(pl(s3, 1), pl(cAll, 3), pl(aAll, 3), AL.mult)
    TT(pl(s3, 2), pl(s3, 0), pl(s3, 1), AL.add)
    TT(pl(s3, 0), pl(cAll, 4), pl(aAll, 4), AL.mult)
    # final det add writes the fp32 recip input directly (skips the ACT
    # up-cast hop; mixed-dtype drops this one [1,K] op to 1x, still a win)
    TT(i3f, pl(s3, 2), pl(s3, 0), AL.add)              # i3, fp32

    # That off-diag products that don't need e8a00/r3: they keep DVE busy
    # while ACT runs the e8 scaled copies and the reciprocal resolves.
    TT(pl(pO, 3), pl(aAll, 3), pl(aAll, 1), AL.mult)   # a01 a11
    TT(pl(pO, 4), pl(aAll, 3), pl(aAll, 5), AL.mult)   # a01 a12
    TT(pl(pO, 5), pl(aAll, 1), pl(aAll, 5), AL.mult)   # a11 a12
    TT(pl(pO, 6), pl(aAll, 4), pl(aAll, 5), AL.mult)   # a02 a12
    ACT(e8a00, pl(aAll, 0), AF.Copy, scale=8.0)
    ACT(e8a01, pl(aAll, 3), AF.Copy, scale=8.0)
    nc.vector.reciprocal_approx_fast(i3f, i3f)         # 1/I3, in place
    TT(pl(pO, 7), pl(aAll, 4), pl(aAll, 2), AL.mult)   # a02 a22
    TT(pl(pO, 8), pl(aAll, 5), pl(aAll, 2), AL.mult)   # a12 a22
    TT(pl(pO, 0), e8a00, pl(aAll, 3), AL.mult)         # 8 a00 a01
    TT(pl(pO, 1), e8a00, pl(aAll, 4), AL.mult)         # 8 a00 a02
    TT(pl(pO, 2), e8a01, pl(aAll, 4), AL.mult)         # 8 a01 a02

    # ---- t3 = 2 I5, kappa (xk), lambda (lam) -----------------------------
    # scalar_tensor_tensor (1x-only, fine at [1,K]) fuses the -56 bias and
    # the -0.2 scale and reads the fp32 reciprocal directly, cutting three
    # serial ACT hops out of the critical path.
    STT = nc.vector.scalar_tensor_tensor
    TT(pl(s3, 0), e8a00, pl(aAll, 1), AL.add)
    TT(t3, pl(s3, 0), pl(aAll, 2), AL.add)
    ACT(pl(s3, 0), t3, AF.Square, scale=SQRT02)        # 0.2 t3^2
    STT(pl(sf, 6), pl(s3, 0), -56.0, i3f, AL.add, AL.mult)   # (0.2t3^2-56) r3
    ACT(xk, pl(sf, 6), AF.Copy, bias=20.0)
    STT(lam, t3, -0.2, i3f, AL.mult, AL.mult)          # -0.2 t3 r3

    # diag 8 A_i0^2 prefetch, then the bulk sqa
    ACT(pl(s3, 0), pl(aAll, 0), AF.Square, scale=SQRT8)   # 8 a00^2
    ACT(pl(s3, 1), pl(aAll, 3), AF.Square, scale=SQRT8)   # 8 a01^2
    ACT(pl(s3, 2), pl(aAll, 4), AF.Square, scale=SQRT8)   # 8 a02^2
    ACT(pl(sf, 0, 6), pl(aAll, 0, 6), AF.Square)       # sqa

    # ---- That accumulation (into cAll slots; C is dead; sf[6:9] is the
    # temp since s3 now carries the th8sq prefetch) -------------------------
    TT(sf[:, 6:9, :], pl(pO, 0, 3), pl(pO, 3, 3), AL.add)
    TT(cAll[:, 3:6, :], sf[:, 6:9, :], pl(pO, 6, 3), AL.add)  # (th01,th02,th12)
    # diag: 8 A_i0^2 + A_i1^2 + A_i2^2 from sqa + scaled squares
    for (i, q1, q2) in ((0, 3, 4), (1, 1, 5), (2, 5, 2)):
        TT(pl(pO, 0), pl(s3, i), pl(sf, q1), AL.add)
        TT(pl(cAll, i), pl(pO, 0), pl(sf, q2), AL.add)

    # ---- S = xk A + lam That + diag(g) -----------------------------------
    TT(pl(sf, 0, 6), pl(aAll, 0, 6), bc(xk, 6), AL.mult)    # k1 (sqa dead)
    TT(pl(pO, 0, 6), pl(cAll, 0, 6), bc(lam, 6), AL.mult)   # k2
    TT(pl(aAll, 0, 6), pl(sf, 0, 6), pl(pO, 0, 6), AL.add)  # S -> aAll
    ACT(pl(s3, 0), t2b, AF.Copy, scale=1.6, bias=16.0)      # g0
    ACT(pl(s3, 1, 2), bc(t2b, 2), AF.Copy, scale=0.2, bias=16.0)  # g12
    TT(pl(sf, 0, 3), pl(aAll, 0, 3), s3, AL.add)            # S diag + g

    # ---- P = F S  (S symmetric; diag in sf[0:3], off-diag in aAll[3:6]) --
    Sv = [[pl(sf, 0), pl(aAll, 3), pl(aAll, 4)],
          [pl(aAll, 3), pl(sf, 1), pl(aAll, 5)],
          [pl(aAll, 4), pl(aAll, 5), pl(sf, 2)]]
    for j in range(3):
        TT(pl(pO, 0, 3), fc[:, 0:9:3, :], bc(Sv[0][j], 3), AL.mult)
        TT(pl(pO, 3, 3), fc[:, 1:9:3, :], bc(Sv[1][j], 3), AL.mult)
        TT(pl(pO, 6, 3), pl(pO, 0, 3), pl(pO, 3, 3), AL.add)
        TT(pl(pO, 0, 3), fc[:, 2:9:3, :], bc(Sv[2][j], 3), AL.mult)
        TT(pc[:, j:9:3, :], pl(pO, 6, 3), pl(pO, 0, 3), AL.add)


def _build():
    import concourse.bass as bass
    import concourse.tile as tile
    from concourse import bacc, mybir
    from contextlib import ExitStack

    f16 = mybir.dt.float16
    f32 = mybir.dt.float32
    AL = mybir.AluOpType
    AF = mybir.ActivationFunctionType

    nc = bacc.Bacc("TRN2", target_bir_lowering=False, debug=False)
    fin_d = nc.dram_tensor("fin", [PART, NT, 9, K], f16, kind="ExternalInput").ap()
    pout_d = nc.dram_tensor("pout", [PART, NT, 9, K], f16, kind="ExternalOutput").ap()

    with tile.TileContext(nc) as tc:
        with ExitStack() as ctx:
            io = ctx.enter_context(tc.tile_pool(name="io", bufs=2))
            sp = ctx.enter_context(tc.tile_pool(name="sp", bufs=1))

            # Issue all input DMAs up front: the tile-t+1 load must not queue
            # behind the tile-t store's semaphore wait on the SP sequencer.
            # Column-group granularity so the first C product (cols 0,1)
            # starts before the full tile has landed.
            fcs = []
            for t in range(NT):
                ft = io.tile([PART, 9, K], f16, name="fin", tag="fin", bufs=2)
                for c in range(3):
                    nc.sync.dma_start(ft[:, c:9:3, :], fin_d[:, t, c:9:3, :])
                fcs.append(ft)

            for t in range(NT):
                pc = io.tile([PART, 9, K], f16, name="pout", tag="pout",
                             bufs=1)
                _emit_tile(nc, sp, fcs[t], pc, AL, AF, f16, f32)
                # per-column stores: P column j is complete as soon as its
                # FS pass finishes, so the tail is one column, not the tile
                for j in range(3):
                    nc.sync.dma_start(pout_d[:, t, j:9:3, :], pc[:, j:9:3, :])

    nc.compile()
    return nc


def _get_nc():
    if "nc" not in _cache:
        _cache["nc"] = _build()
    return _cache["nc"]


def _make_in_maps(F):
    x = F.reshape(N, 9).astype(np.float16)
    eye9 = np.array([1, 0, 0, 0, 1, 0, 0, 0, 1], dtype=np.float16)
    pad = np.tile(eye9, (NPADPC - NPC, 1))
    in_maps = []
    for cidx in range(NCORES):
        xc = x[cidx * NPC:(cidx + 1) * NPC]
        xcp = (np.concatenate([xc, pad], axis=0)
               .reshape(PART, NT, K, 9).transpose(0, 1, 3, 2))
        in_maps.append({"fin": np.ascontiguousarray(xcp)})
    return in_maps


def kernel(**inputs):
    from concourse.bass_utils import run_bass_kernel_spmd

    F = np.asarray(inputs["F"], dtype=np.float32)
    nc = _get_nc()
    in_maps = _make_in_maps(F)

    res = run_bass_kernel_spmd(nc, in_maps, list(range(NCORES)))

    out = np.empty((N, 9), dtype=np.float32)
    for cidx in range(NCORES):
        oc = (np.asarray(res.results[cidx]["pout"]).astype(np.float32)
              .reshape(PART, NT, 9, K).transpose(0, 1, 3, 2)
              .reshape(NPADPC, 9))
        out[cidx * NPC:(cidx + 1) * NPC] = oc[:NPC]
    return out.reshape(N, 3, 3)
